# revision 27
# baseline (speedup 1.0000x reference)
"""Trainium2 Bass kernel: 12-head attention (B=2, N=2048, C=768) on 8 NeuronCores.

Sharding: core c -> batch b = c // 4, head-group g = c % 4 (heads 3g..3g+2).

Structure (v2 — collective/pstate-aware schedule):

- Mask compaction: tokens of each batch are permuted so kept keys come first;
  K/V/attention only process ceil(max_kept/128)*128 keys. Host un-permutes.

- Head packing on the PE array: wq/wk laid out [q0|q1] / [k0|k1] so the two
  heads' QK matmuls (contraction 64) occupy disjoint row-halves of the array
  and stream concurrently; head 2 uses duplicated columns and alternates
  halves chunk-to-chunk.

- A tiny dummy AllToAll fires at kernel start so the all-core rendezvous /
  launch skew is absorbed while the input DMAs and QKV projections run;
  the four real A2As then pay only wire time.

- The PE clock ramps (0.65 -> 1.2 -> 2.4 GHz after ~3us continuous busy), so
  junk warm-up matmuls run during the input-DMA wait and the schedule keeps
  the PE fed with fill-in work (V/K/Q projections, out-projection m-tiles)
  inside the scalar-engine-paced attention units.

- Input DMAs are issued in need-order with merged pieces (8 dma_starts; each
  costs ~0.7-1.4us serialized on the Sync queue): xT[0:1024] first (covers
  the K bootstrap and Q half-0), then wk, wq, mf, wv, xT[1024:2048], wp, bp.

- Softmax denominator via an extra all-ones column appended to V; the
  denominator row is broadcast via a rank-1 PE matmul, then DVE reciprocal
  + multiply.

- Out-projection for half 0 runs as fill-in inside h2(1) (its A2As complete
  long before); for half 1 it is split into an a-part (wp chunks 0-3, runs
  right after bounce_b(1) under the in-flight last A2A, partials+bias parked
  in SBUF) and a b-part (chunks 4-5 + partial add) so only ~2us of matmul
  work remains after the final collective lands. Output is written bf16.
"""

import numpy as np
import ml_dtypes

B, N, C = 2, 2048, 768
H, HD = 12, 64
HPG = 3            # heads per core
GPB = 4            # cores (head-groups) per batch
NCORES = 8
SCALE = float(HD) ** -0.5
DCH = C // 128     # 6 contraction chunks
NH = N // 2        # 1024 query tokens per half

bf = ml_dtypes.bfloat16

_cache = {}


def _build(nkch):
    import concourse.mybir as mybir
    import concourse.tile as tile
    from concourse import bacc

    fp32 = mybir.dt.float32
    bfl = mybir.dt.bfloat16
    EXP = mybir.ActivationFunctionType.Exp
    MULT = mybir.AluOpType.mult
    ADD = mybir.AluOpType.add

    NK = nkch * 128  # padded key count

    nc = bacc.Bacc(None, num_devices=NCORES)
    xT = nc.declare_dram_parameter("xT", [C, N], bfl, isOutput=False)
    wq = nc.declare_dram_parameter("wq", [C, 256], bfl, isOutput=False)
    wk = nc.declare_dram_parameter("wk", [C, 256], bfl, isOutput=False)
    wv = nc.declare_dram_parameter("wv", [C, HPG * HD], bfl, isOutput=False)
    wp = nc.declare_dram_parameter("wp", [C, C], bfl, isOutput=False)
    bp = nc.declare_dram_parameter("bp", [128, DCH], fp32, isOutput=False)
    mf = nc.declare_dram_parameter("mf", [128, nkch], fp32, isOutput=False)
    out = nc.declare_dram_parameter("out", [C, 512], bfl, isOutput=True)

    # zero payload for the rendezvous-absorbing warm-up collective
    warm_cc_in = nc.inline_tensor(np.zeros((8, 16), dtype=bf), name="warm_cc_in")
    warm_cc_out = nc.dram_tensor("warm_cc_out", [8, 16], bfl)

    with tile.TileContext(nc) as tc:
        with (
            tc.tile_pool(name="const", bufs=1) as cpool,
            tc.tile_pool(name="work", bufs=1) as wpool,
            tc.tile_pool(name="pp", bufs=2) as ppool,
        ):
            # ---- warm-up collective: absorbs the all-core rendezvous ----
            nc.gpsimd.collective_compute(
                "AllToAll",
                mybir.AluOpType.bypass,
                replica_groups=[[0, 1, 2, 3, 4, 5, 6, 7]],
                ins=[warm_cc_in[:].opt()],
                outs=[warm_cc_out[:].opt()],
            )

            # ---------------- input loads (order = need order) ----------------
            xT_sb = cpool.tile([128, DCH, N], bfl, tag="xT")
            xT_r = xT.rearrange("(o p) t -> p o t", p=128)
            nc.sync.dma_start(xT_sb[:, :, 0:NH], xT_r[:, :, 0:NH])
            wk_sb = cpool.tile([128, DCH, 256], bfl, tag="wk")
            nc.sync.dma_start(wk_sb[:], wk.rearrange("(o p) c -> p o c", p=128))
            wq_sb = cpool.tile([128, DCH, 256], bfl, tag="wq")
            nc.sync.dma_start(wq_sb[:], wq.rearrange("(o p) c -> p o c", p=128))
            mf_sb = cpool.tile([128, nkch], fp32, tag="mf")
            nc.sync.dma_start(mf_sb[:], mf[:])
            wv_sb = cpool.tile([128, DCH, HPG * HD], bfl, tag="wv")
            nc.sync.dma_start(wv_sb[:], wv.rearrange("(o p) c -> p o c", p=128))
            nc.sync.dma_start(xT_sb[:, :, NH:N], xT_r[:, :, NH:N])
            wp_sb = cpool.tile([128, DCH, C], bfl, tag="wp")
            nc.sync.dma_start(wp_sb[:], wp.rearrange("(o p) c -> p o c", p=128))
            bp_sb = cpool.tile([128, DCH], fp32, tag="bp")
            nc.sync.dma_start(bp_sb[:], bp[:])

            # preload the exp table set + constants while DMAs run
            warm = cpool.tile([1, 8], fp32, tag="warm")
            nc.vector.memset(warm[:], 0.0)
            nc.scalar.activation(warm[:], warm[:], EXP)
            ones_sb = cpool.tile([128, 64], bfl, tag="ones")
            nc.vector.memset(ones_sb[:], 1.0)
            wsrc = cpool.tile([128, 512], bfl, tag="wsrc")
            nc.vector.memset(wsrc[:], 0.0)

            qs = wpool.tile([128, N], bfl, tag="qs")      # [q0 | q1] channel-major
            qs2 = wpool.tile([128, N], bfl, tag="qs2")    # [q2 | q2]
            kst = wpool.tile([128, 2, NK], bfl, tag="kst")  # [:,0]=[k0|k1] [:,1]=[k2|k2]
            V3 = wpool.tile([128, nkch, HPG, HD + 1], bfl, tag="V3")
            pa = wpool.tile([128, DCH, 256], bfl, tag="pa")  # proj1 a-part partials

            # PSUM: tag "s" 2 slots x 2 banks (QK scores + all projection /
            # fill-in tiles), tag "o" 2 slots x 2 banks (live PV accumulators).
            sps_cm = tc.tile_pool(name="sps", bufs=2, space="PSUM")
            sps = sps_cm.__enter__()
            ops_cm = tc.tile_pool(name="ops", bufs=2, space="PSUM")
            ops = ops_cm.__enter__()

            def warm_fill(n):
                """Junk matmuls that ramp/hold the PE clock."""
                for _ in range(n):
                    w_ps = sps.tile([128, NH], fp32, tag="s", name="w_ps")[:, :512]
                    nc.tensor.matmul(
                        w_ps[:], lhsT=wsrc[:, 0:128], rhs=wsrc[:],
                        start=True, stop=True,
                    )

            # Ramp matmuls bridging the input-DMA wait (the other cores are
            # still in their launch stagger, so shared power is available and
            # these keep the PE clock from dropping before the bootstrap).
            # Elsewhere junk matmuls BURN the fleet's shared power budget and
            # queue ahead of real work — use none.
            warm_fill(24)

            def qk_pass(which, m, lo, wid):
                """Q or K projection Mtile m over token range [lo, lo+wid)."""
                w_sb = wq_sb if which == "q" else wk_sb
                t = sps.tile([128, NH], fp32, tag="s", name="qk_t")[:, :wid]
                for kk in range(DCH):
                    nc.tensor.matmul(
                        t[:],
                        lhsT=w_sb[:, kk, m * 128 : (m + 1) * 128],
                        rhs=xT_sb[:, kk, lo : lo + wid],
                        start=(kk == 0),
                        stop=(kk == DCH - 1),
                    )
                if which == "q":
                    dst = qs if m == 0 else qs2
                    nc.vector.tensor_copy(dst[:, lo : lo + wid], t[:])
                else:
                    nc.vector.tensor_copy(kst[:, m, lo : lo + wid], t[:])

            def v_pass(c):
                """V projection for key chunk c -> V3 (values * mf, ones col)."""
                v_t = sps.tile([128, NH], fp32, tag="s", name="v_t")[:, : HPG * HD]
                for kk in range(DCH):
                    nc.tensor.matmul(
                        v_t[:],
                        lhsT=xT_sb[:, kk, c * 128 : (c + 1) * 128],
                        rhs=wv_sb[:, kk, :],
                        start=(kk == 0),
                        stop=(kk == DCH - 1),
                    )
                nc.vector.tensor_scalar_mul(
                    V3[:, c, :, 0:HD],
                    v_t[:].rearrange("p (h d) -> p h d", h=HPG),
                    mf_sb[:, c : c + 1],
                )
                nc.vector.tensor_copy(
                    V3[:, c, :, HD], mf_sb[:, c : c + 1].to_broadcast((128, HPG))
                )

            OnA = [wpool.tile([128, NH], bfl, tag=f"OnA{q}", name=f"OnA{q}") for q in range(2)]
            OnB = [wpool.tile([64, NH], bfl, tag=f"OnB{q}", name=f"OnB{q}") for q in range(2)]

            def normalize(heads, qh, o_ts):
                """osb <- o in bf16 for every head first (frees the PV PSUM
                accumulators fast); the denominator rows are then broadcast to
                64 partitions via rank-1 bf16 PE matmuls into recycled "o"
                slots (NOT the "s" ring — an s-tag rbb would make the next
                unit's QKs wait on this DVE chain), then DVE recip+multiply."""
                osbs = []
                for h, o_t in zip(heads, o_ts):
                    osb = wpool.tile([HD + 1, NH], bfl, tag="osb", bufs=2, name="osb")
                    nc.vector.tensor_copy(osb[:], o_t[:])
                    osbs.append(osb)
                rbbs = []
                for h, osb in zip(heads, osbs):
                    rbb = ops.tile([HD + 1, NH], fp32, tag="o", name="rbb")[0:HD, :]
                    for n2 in range(2):
                        nc.tensor.matmul(
                            rbb[:, n2 * 512 : (n2 + 1) * 512],
                            lhsT=ones_sb[HD : HD + 1, :],
                            rhs=osb[HD : HD + 1, n2 * 512 : (n2 + 1) * 512],
                            start=True,
                            stop=True,
                        )
                    rbbs.append(rbb)
                for h, osb, rbb in zip(heads, osbs, rbbs):
                    rb = wpool.tile([HD, NH], fp32, tag="rb", bufs=2, name="rb")
                    nc.vector.reciprocal_approx_fast(rb[:], rbb[:])
                    dst = OnA[qh][h * 64 : (h + 1) * 64, :] if h < 2 else OnB[qh][:, :]
                    # pair-head multiplies go to the idle GpSimd engine so the
                    # DVE is free for the next unit's fill copies; the
                    # terminal h2 multiply stays on the (faster) DVE.
                    eng = nc.gpsimd if h < 2 else nc.vector
                    eng.tensor_tensor(dst, osb[0:HD, :], rb[:], MULT)

            def qk_mm(s_t, ksrc, qsrc, base, c, qh):
                for n2 in range(2):
                    nc.tensor.matmul(
                        s_t[:, n2 * 512 : (n2 + 1) * 512],
                        lhsT=ksrc[base : base + 64, c * 128 : (c + 1) * 128],
                        rhs=qsrc[
                            base : base + 64,
                            qh * NH + n2 * 512 : qh * NH + (n2 + 1) * 512,
                        ],
                        start=True,
                        stop=True,
                    )

            def pv_mm(o_t, p_t, c, h):
                for n2 in range(2):
                    nc.tensor.matmul(
                        o_t[:, n2 * 512 : (n2 + 1) * 512],
                        lhsT=V3[:, c, h, :],
                        rhs=p_t[:, c, n2 * 512 : (n2 + 1) * 512],
                        start=(c == 0),
                        stop=(c == nkch - 1),
                    )

            def run_ex(f):
                if f is None:
                    return
                if isinstance(f, (list, tuple)):
                    for g in f:
                        g()
                else:
                    f()

            def pair_unit(qh, extras=(), tail_extras=()):
                """Heads 0+1, chunk-interleaved, query half qh. PV for chunk
                c-1 is emitted after chunk c's QK (FIFO queue stays unblocked).
                extras: thunks inserted one per chunk (fill-in projections).
                tail_extras: emitted after the last PV but BEFORE normalize so
                the next unit's K/Q copies aren't queued behind the
                normalize's DVE chain."""
                p_t = [
                    ppool.tile([128, nkch, NH], bfl, tag="p", name=f"pu{h}")
                    for h in range(2)
                ]
                o_t = [ops.tile([HD + 1, NH], fp32, tag="o", name=f"ou{h}") for h in range(2)]
                ex = list(extras)
                for c in range(nkch):
                    if c < len(ex):
                        run_ex(ex[c])
                    s_t = []
                    for h in range(2):
                        st = sps.tile([128, NH], fp32, tag="s", name=f"s{h}")
                        qk_mm(st, kst[:, 0], qs, 64 * h, c, qh)
                        s_t.append(st)
                    for h in range(2):
                        nc.scalar.activation(p_t[h][:, c, :], s_t[h][:], EXP, scale=SCALE)
                    if c > 0:
                        for h in range(2):
                            pv_mm(o_t[h], p_t[h], c - 1, h)
                for h in range(2):
                    pv_mm(o_t[h], p_t[h], nkch - 1, h)
                for f in tail_extras:
                    run_ex(f)
                normalize([0, 1], qh, o_t)

            def h2_unit(qh, extras=(), tail_extras=()):
                """Head 2 over query half qh; kst[:,1]/qs2 hold [k2|k2]/[q2|q2]
                so chunks alternate array row-halves."""
                p_t = ppool.tile([128, nkch, NH], bfl, tag="p", name="pu2")
                o_t = ops.tile([HD + 1, NH], fp32, tag="o", name="ou2")
                ex = list(extras)
                for c in range(nkch):
                    if c < len(ex):
                        run_ex(ex[c])
                    s_t = sps.tile([128, NH], fp32, tag="s", name="s2")
                    qk_mm(s_t, kst[:, 1], qs2, 64 * (c % 2), c, qh)
                    nc.scalar.activation(p_t[:, c, :], s_t[:], EXP, scale=SCALE)
                    if c > 0:
                        pv_mm(o_t, p_t, c - 1, 2)
                pv_mm(o_t, p_t, nkch - 1, 2)
                for f in tail_extras:
                    run_ex(f)
                normalize([2], qh, [o_t])

            ag_a_in = [
                nc.dram_tensor(f"ag_a_in{q}", [NCORES * 128, 128], bfl)
                for q in range(2)
            ]
            ag_a_out = [
                nc.dram_tensor(f"ag_a_out{q}", [NCORES * 128, 128], bfl)
                for q in range(2)
            ]
            ag_b_in = [
                nc.dram_tensor(f"ag_b_in{q}", [NCORES * 64, 128], bfl)
                for q in range(2)
            ]
            ag_b_out = [
                nc.dram_tensor(f"ag_b_out{q}", [NCORES * 64, 128], bfl)
                for q in range(2)
            ]

            def bounce_a(qh):
                nc.sync.dma_start(
                    ag_a_in[qh].rearrange("(j p) t -> p j t", j=NCORES),
                    OnA[qh][:, :].rearrange("p (j t) -> p j t", j=NCORES),
                )

            def bounce_b(qh):
                nc.sync.dma_start(
                    ag_b_in[qh].rearrange("(j p) t -> p j t", j=NCORES),
                    OnB[qh][:, :].rearrange("p (j t) -> p j t", j=NCORES),
                )

            def a2a(qh, part):
                agi, ago = (ag_a_in, ag_a_out) if part == 0 else (ag_b_in, ag_b_out)
                nc.gpsimd.collective_compute(
                    "AllToAll",
                    mybir.AluOpType.bypass,
                    replica_groups=[[0, 1, 2, 3, 4, 5, 6, 7]],
                    ins=[agi[qh][:].opt()],
                    outs=[ago[qh][:].opt()],
                )

            out_r = out.rearrange("(o p) t -> p o t", p=128)

            def proj_dma_a(qh):
                """Gather the a-part (wp chunks 0-3) of the A2A'd half."""
                at_sb = wpool.tile(
                    [128, 2, DCH, 128], bfl, tag=f"at{qh}", bufs=1, name=f"at_sb{qh}"
                )
                for b in range(2):
                    nc.sync.dma_start(
                        at_sb[:, b, 0:4, :],
                        ag_a_out[qh][b * 512 : (b + 1) * 512, :].rearrange(
                            "(o p) t -> p o t", p=128
                        ),
                    )
                return at_sb

            def proj_dma_b(qh, at_sb):
                for b in range(2):
                    nc.sync.dma_start(
                        at_sb[:, b, 4:6, :],
                        ag_b_out[qh][b * 256 : (b + 1) * 256, :].rearrange(
                            "(o p) t -> p o t", p=128
                        ),
                    )

            def proj_mm_group(at_sb, ms, k0, k1):
                """Projection m-tiles `ms` (<=4) packed as 256-col quarters of
                ONE psum s-slot; each quarter is its own accumulation group."""
                y_ps = sps.tile([128, NH], fp32, tag="s", name="y_ps")
                for i, m in enumerate(ms):
                    dst = y_ps[:, i * 256 : (i + 1) * 256]
                    for kk in range(k0, k1):
                        nc.tensor.matmul(
                            dst.rearrange("p (b t) -> p b t", b=2),
                            lhsT=wp_sb[:, kk, m * 128 : (m + 1) * 128],
                            rhs=at_sb[:, :, kk, :],
                            start=(kk == k0),
                            stop=(kk == k1 - 1),
                        )
                return y_ps

            def proj_full(qh, at_sb, ydst):
                """Full 6-chunk projection of half qh into ydst [128,6,256]."""
                for ms in ([0, 1, 2, 3], [4, 5]):
                    y_ps = proj_mm_group(at_sb, ms, 0, DCH)
                    for i, m in enumerate(ms):
                        nc.vector.tensor_scalar_add(
                            ydst[:, m, :],
                            y_ps[:, i * 256 : (i + 1) * 256],
                            bp_sb[:, m : m + 1],
                        )
                nc.sync.dma_start(out_r[:, :, qh * 256 : (qh + 1) * 256], ydst[:])

            def proj1_a(at_sb):
                """Half-1 a-part: wp chunks 0-3, partials+bias parked in pa."""
                for ms in ([0, 1, 2, 3], [4, 5]):
                    y_ps = proj_mm_group(at_sb, ms, 0, 4)
                    for i, m in enumerate(ms):
                        nc.vector.tensor_scalar_add(
                            pa[:, m, :],
                            y_ps[:, i * 256 : (i + 1) * 256],
                            bp_sb[:, m : m + 1],
                        )

            def proj1_b(at_sb, ydst):
                """Half-1 b-part: wp chunks 4-5 + parked partials (which
                already carry the bias), merged epilogue adds, then out."""
                for ms in ([0, 1, 2, 3], [4, 5]):
                    y_ps = proj_mm_group(at_sb, ms, 4, DCH)
                    nc.vector.tensor_tensor(
                        ydst[:, ms[0] : ms[-1] + 1, :],
                        y_ps[:, : len(ms) * 256].rearrange(
                            "p (m t) -> p m t", m=len(ms)
                        ),
                        pa[:, ms[0] : ms[-1] + 1, :],
                        ADD,
                    )
                nc.sync.dma_start(out_r[:, :, 256:512], ydst[:])

            # ---------------- schedule ----------------
            # Units run h2-first per half: an h2 chunk makes only ONE psum
            # "s"-ring allocation, so it can host a fill (2 allocs/chunk on
            # the 2-deep ring) without serializing QKs behind exps; pair
            # chunks already make 2 allocs and run fill-free.
            def kslice(m, lo, hi):
                return lambda: qk_pass("k", m, lo, min(hi, NK) - lo)

            # Bootstrap: h2(0) needs kst[:,1] keys 0:512 and qs2[:, 0:1024].
            qk_pass("k", 1, 0, 512)
            qk_pass("q", 1, 0, 512)
            qk_pass("q", 1, 512, 512)

            vs = [lambda c=c: v_pass(c) for c in range(nkch)]
            # h2(0) fills: v(c) before chunk c+1's PV (slot c+1, so a late
            # wv/mf DMA cannot head-of-line block the first exps); kst[:,1]
            # keys 512:1024 by chunk 4, 1024:NK by chunk 8; pair(0)'s K m0
            # keys 0:512 + Q half-0 before pair(0) starts.
            ex_h0 = [
                None, vs[0], vs[1], [vs[2], kslice(1, 512, 1024)], vs[3],
                vs[4], [vs[5], kslice(0, 0, 512)],
                [vs[6], kslice(1, 1024, 1280)], [vs[7], vs[8]],
            ]
            tail_h0 = [
                lambda: qk_pass("q", 0, 0, 512),
                lambda: qk_pass("q", 0, 512, 512),
            ]
            h2_unit(0, extras=ex_h0[:nkch], tail_extras=tail_h0)
            bounce_b(0)
            a2a(0, 1)

            ex_p0 = [
                None, None, None, kslice(0, 512, 1024), None,
                None, None, kslice(0, 1024, 1280), None,
            ]
            tail_p0 = [
                lambda: qk_pass("q", 0, 1024, 512),
                lambda: qk_pass("q", 0, 1536, 512),
            ]
            pair_unit(0, extras=ex_p0[:nkch], tail_extras=tail_p0)
            bounce_a(0)
            a2a(0, 0)

            # ---- half 1: the half-0 collectives run under pair(1)/h2(1).
            tail_p1 = [
                lambda: qk_pass("q", 1, 1024, 512),
                lambda: qk_pass("q", 1, 1536, 512),
            ]
            pair_unit(1, tail_extras=tail_p1)
            bounce_a(1)
            a2a(1, 0)

            h2_unit(1)
            bounce_b(1)
            a2a(1, 1)

            # ---- tail: proj(0) (its A2As are long done) + proj(1)-a run
            # under the in-flight last collectives; only proj(1)-b + one DMA
            # remain after a2a(1,1) lands.
            y0 = wpool.tile([128, DCH, 256], bfl, tag="y0", bufs=1, name="y0")
            y1 = wpool.tile([128, DCH, 256], bfl, tag="y1", bufs=1, name="y1")
            at0 = proj_dma_a(0)
            proj_dma_b(0, at0)
            proj_full(0, at0, y0)
            at1 = proj_dma_a(1)
            proj1_a(at1)
            warm_fill(4)
            proj_dma_b(1, at1)
            proj1_b(at1, y1)

            ops_cm.__exit__(None, None, None)
            sps_cm.__exit__(None, None, None)

    nc.finalize()
    return nc


def _prep(x, mask, w_qkv, w_proj, b_proj):
    """Host-side compaction: per-batch token permutation (kept keys first) and
    per-core input shards."""
    perms, counts = [], []
    for b in range(B):
        perm = np.argsort(1 - mask[b], kind="stable")
        perms.append(perm)
        counts.append(int(mask[b].sum()))
    nkch = max(1, int(np.ceil(max(counts) / 128)))
    NK = nkch * 128

    xTs = []
    mfs = []
    for b in range(B):
        xp = np.ascontiguousarray(x[b][perms[b]].T).astype(bf)
        xTs.append(xp)
        m = np.zeros(NK, dtype=np.float32)
        m[: counts[b]] = 1.0
        mfs.append(np.ascontiguousarray(m.reshape(nkch, 128).T))

    bp_t = np.ascontiguousarray(b_proj.astype(np.float32).reshape(DCH, 128).T)
    # wp rows permuted to the split-A2A arrival order: slots 0-3 carry heads
    # (3g, 3g+1) of group g, slots 4-5 carry heads 3g+2.
    perm_rows = np.empty(C, dtype=np.int64)
    for o in range(4):
        for p in range(128):
            perm_rows[o * 128 + p] = (3 * o + p // 64) * 64 + (p % 64)
    for idx in range(256):
        perm_rows[512 + idx] = (3 * (idx // 64) + 2) * 64 + (idx % 64)
    wp_t = np.ascontiguousarray(w_proj[perm_rows]).astype(bf)

    in_maps = []
    for c in range(NCORES):
        b, g = c // GPB, c % GPB
        heads = [3 * g, 3 * g + 1, 3 * g + 2]
        q_cols = [h * HD + d for h in (heads[0], heads[1], heads[2], heads[2]) for d in range(HD)]
        k_cols = [C + h * HD + d for h in (heads[0], heads[1], heads[2], heads[2]) for d in range(HD)]
        v_cols = [2 * C + h * HD + d for h in heads for d in range(HD)]
        in_maps.append(
            {
                "xT": xTs[b],
                "wq": np.ascontiguousarray(w_qkv[:, q_cols]).astype(bf),
                "wk": np.ascontiguousarray(w_qkv[:, k_cols]).astype(bf),
                "wv": np.ascontiguousarray(w_qkv[:, v_cols]).astype(bf),
                "wp": wp_t,
                "bp": bp_t,
                "mf": mfs[b],
            }
        )
    return in_maps, perms, nkch


def kernel(x, mask, w_qkv, w_proj, b_proj, _trace=False):
    from concourse.bass_utils import run_bass_kernel_spmd

    x = np.asarray(x, dtype=np.float32)
    mask = np.asarray(mask)
    w_qkv = np.asarray(w_qkv, dtype=np.float32)
    w_proj = np.asarray(w_proj, dtype=np.float32)
    b_proj = np.asarray(b_proj, dtype=np.float32)
    in_maps, perms, nkch = _prep(x, mask, w_qkv, w_proj, b_proj)
    if ("nc", nkch) not in _cache:
        _cache[("nc", nkch)] = _build(nkch)
    nc = _cache[("nc", nkch)]
    res = run_bass_kernel_spmd(nc, in_maps, core_ids=list(range(NCORES)), trace=_trace)
    y = np.empty((B, N, C), dtype=np.float32)
    for c in range(NCORES):
        o = np.asarray(res.results[c]["out"]).astype(np.float32)
        for qh in range(2):
            base = qh * NH + c * 128
            for b in range(B):
                y[b, perms[b][base : base + 128]] = o[
                    :, qh * 256 + b * 128 : qh * 256 + (b + 1) * 128
                ].T
    if _trace:
        _cache["last_exec_time_ns"] = res.exec_time_ns
        _cache["last_profile"] = res.profile_json
    return y
